# revision 4
# baseline (speedup 1.0000x reference)
import sys

sys.path.insert(0, "/opt/trn_rl_repo")

import os
import numpy as np
import ml_dtypes

import concourse.bass as bass
import concourse.mybir as mybir
import concourse.tile as tile
from concourse import bacc
from concourse.bass_utils import run_bass_kernel_spmd
from concourse.masks import make_identity

B, S, D, H = 4, 4096, 1024, 64
QW = 512                      # q-chunk width
NQ = 4                        # q-chunk slots per core
POS = [(0, 3, 4, 7), (1, 2, 5, 6)]   # q-chunk positions per core class
T = (8, 16, 24, 32)           # k-tiles (128 wide) per slot in the uniform graph
NKT = S // 128                # 32 k tiles
ND = D // 128                 # 8 d-tiles
DIAG_JG = (0, 3, 4, 7)        # diagonal 512-group per slot (class-independent)
OOB_JG = (1, 2, 5, 6)         # maybe-out-of-bounds group per slot
OOB_LC = {1: 0, 2: 1, 5: 2, 6: 3}   # chunk -> local index in vws2

BF = mybir.dt.bfloat16
F32 = mybir.dt.float32

_cache = {}


def _build():
    nc = bacc.Bacc("TRN2", target_bir_lowering=False, debug=False, num_devices=8)

    # host pre-swizzled inputs: partition-major layouts so each load is ONE DMA
    xt3 = nc.dram_tensor("xt3", [128, ND, S], BF, kind="ExternalInput").ap()
    # slot-1/2/3 q-chunk blocks, loaded early so no qproj waits on the
    # main chunk stream
    qx = nc.dram_tensor("qx", [128, ND, 3 * QW], BF, kind="ExternalInput").ap()
    wall = nc.dram_tensor("wall", [128, ND * 256], BF, kind="ExternalInput").ap()
    # per-tile validity scalars for the maybe-OOB groups (chunks 1,2,5,6)
    misc = nc.dram_tensor("misc", [128, 16], F32, kind="ExternalInput").ap()
    o = nc.dram_tensor("o", [NQ, H + 1, QW], BF, kind="ExternalOutput").ap()

    with tile.TileContext(nc) as tc:
        with (
            tc.tile_pool(name="persist", bufs=1) as pp,
            tc.tile_pool(name="xin", bufs=1) as xp,
            tc.tile_pool(name="estage", bufs=4) as ep,
            tc.tile_pool(name="vstage", bufs=3) as vsp,
            tc.tile_pool(name="ostage", bufs=2) as osp,
            tc.tile_pool(name="zpsum", bufs=2, space="PSUM") as zp,
            tc.tile_pool(name="opsum", bufs=1, space="PSUM") as op_,
            tc.tile_pool(name="projpsum", bufs=2, space="PSUM") as prp,
            tc.tile_pool(name="vtpsum", bufs=1, space="PSUM") as vtp,
        ):
            # ---- persistent tiles ----
            wall_sb = pp.tile([128, ND * 256], BF, tag="wall")
            misc_sb = pp.tile([128, 16], F32, tag="misc")
            ident = pp.tile([64, 64], BF, tag="ident")
            qT2 = pp.tile([128, NQ * QW], BF, tag="qT2")
            kT2 = pp.tile([128, S], BF, tag="kT2")
            vws = pp.tile([128, NKT * (H + 1)], BF, tag="vws")
            # zero-or-copy of v for the maybe-OOB groups (chunks 1,2,5,6)
            vws2 = pp.tile([128, 16 * (H + 1)], BF, tag="vws2")
            # mini e-queue: slot-3 pairs 0-3 exp'd early inside attn2's holes
            eq3 = pp.tile([128, 4 * 1024], BF, tag="eq3")
            xtall = xp.tile([128, ND, S], BF, tag="xtall")
            qxall = xp.tile([128, ND, 3 * QW], BF, tag="qxall")

            # ---- input DMAs: one trigger per logical block, consumption order
            nc.sync.dma_start(misc_sb[:], misc[:])
            nc.sync.dma_start(wall_sb[:], wall[:])
            # first two chunks split by d-halves so the kv accumulation can
            # start before the full chunk lands
            nc.sync.dma_start(xtall[:, 0:4, 0:512], xt3[:, 0:4, 0:512])
            nc.sync.dma_start(xtall[:, 4:8, 0:512], xt3[:, 4:8, 0:512])
            nc.sync.dma_start(qxall[:, :, 0:QW], qx[:, :, 0:QW])
            nc.sync.dma_start(xtall[:, 0:4, 512:1024], xt3[:, 0:4, 512:1024])
            nc.sync.dma_start(xtall[:, 4:8, 512:1024], xt3[:, 4:8, 512:1024])
            nc.sync.dma_start(qxall[:, :, QW:3 * QW], qx[:, :, QW:3 * QW])
            for c in range(2, 2 * NQ):
                nc.sync.dma_start(xtall[:, :, c * 512:(c + 1) * 512],
                                  xt3[:, :, c * 512:(c + 1) * 512])

            make_identity(nc, ident[:])
            # hold the PE busy (HAM warm) while the first x chunks stream in
            wtile = vtp.tile([128, 64], F32, tag="vt", name="warmps")
            for i in range(72):
                nc.tensor.matmul(wtile[0:64, :], wall_sb[:, 0:64],
                                 wall_sb[:, 64:128], start=True, stop=True)
            nc.gpsimd.memset(vws[:], 1.0)
            # vws2 ones-row = per-tile validity (0/1 from host)
            nc.gpsimd.memset(vws2[:], 0.0)
            v2ones = vws2[:].rearrange("p (t h) -> p t h", h=H + 1)[:, :, H:H + 1]
            nc.vector.tensor_copy(v2ones, misc_sb[:].rearrange(
                "p (t u) -> p t u", u=1))
            # warm the ACT exp table early
            warm = ep.tile([128, 1], BF, tag="warm")
            nc.scalar.activation(warm[:], misc_sb[:, 0:1],
                                 mybir.ActivationFunctionType.Exp)

            _qps = {}

            def qproj_a(w):
                ps = prp.tile([128, 512], F32, tag="proj", name=f"qps{w}")
                _qps[w] = ps
                for d in range(4):
                    if w >= 1:
                        rhs = qxall[:, d, (w - 1) * QW:w * QW]
                    else:
                        rhs = xtall[:, d, 0:QW]
                    nc.tensor.matmul(ps[:], wall_sb[:, d * 256:d * 256 + 128],
                                     rhs, start=(d == 0), stop=False)

            def qproj_b(w):
                ps = _qps.pop(w)
                for d in range(4, ND):
                    if w >= 1:
                        rhs = qxall[:, d, (w - 1) * QW:w * QW]
                    else:
                        rhs = xtall[:, d, 0:QW]
                    nc.tensor.matmul(ps[:], wall_sb[:, d * 256:d * 256 + 128],
                                     rhs, start=False, stop=(d == ND - 1))
                # rows 0-63 and 64-127 both hold q^T (duplicated weights)
                nc.vector.tensor_copy(qT2[:, w * QW:(w + 1) * QW], ps[:])

            def qproj(w):
                qproj_a(w)
                qproj_b(w)

            _kvps = {}
            _kvvst = {}

            def kv_a(sc):
                ps = prp.tile([128, 512], F32, tag="proj", name=f"kvps{sc}")
                _kvps[sc] = ps
                for d in range(4):
                    nc.tensor.matmul(ps[:], wall_sb[:, d * 256 + 128:d * 256 + 256],
                                     xtall[:, d, sc * 512:(sc + 1) * 512],
                                     start=(d == 0), stop=False)

            def kv_b(sc):
                ps = _kvps.pop(sc)
                for d in range(4, ND):
                    nc.tensor.matmul(ps[:], wall_sb[:, d * 256 + 128:d * 256 + 256],
                                     xtall[:, d, sc * 512:(sc + 1) * 512],
                                     start=False, stop=(d == ND - 1))
                nc.vector.tensor_copy(kT2[0:64, sc * 512:(sc + 1) * 512], ps[0:64, :])
                # duplicate k^T into the upper partition half (for row-tiled QK)
                # via an identity matmul into PE col-group (0,64) — no DMA.
                # kdup borrows the z-pool rotation to keep prp free for proj ps
                kdup = zp.tile([128, 512], F32, tag="z", name=f"kdup{sc}")
                nc.tensor.matmul(kdup[64:128, :], ident[:],
                                 kT2[0:64, sc * 512:(sc + 1) * 512],
                                 start=True, stop=True)
                nc.vector.tensor_copy(kT2[64:128, sc * 512:(sc + 1) * 512],
                                      kdup[64:128, :])
                vstage = vsp.tile([64, 512], BF, tag="vstage", name=f"vst{sc}")
                nc.vector.tensor_copy(vstage[:], ps[64:128, :])
                _kvvst[sc] = vstage

            def kv_c(sc):
                vstage = _kvvst.pop(sc)
                # all 4 transposes into one PSUM tile (68-col stride keeps the
                # matmul writes 8B-aligned), then ONE strided copy into vws
                vt4 = vtp.tile([128, 4 * 68], BF, tag="vt", name=f"vt4_{sc}")
                for t in range(4):
                    nc.tensor.transpose(vt4[:, t * 68:t * 68 + 64],
                                        vstage[:, t * 128:(t + 1) * 128],
                                        ident[:])
                kt0 = 4 * sc
                dst = vws[:, kt0 * (H + 1):(kt0 + 4) * (H + 1)]
                vt4v = vt4[:].rearrange("p (t h) -> p t h", h=68)[:, :, 0:H]
                nc.vector.tensor_copy(
                    dst.rearrange("p (t h) -> p t h", h=H + 1)[:, :, 0:H], vt4v)
                if sc in OOB_LC:
                    # scaled copy into vws2 (valid -> v, invalid -> 0)
                    lc = OOB_LC[sc]
                    d2 = vws2[:, lc * 4 * (H + 1):(lc + 1) * 4 * (H + 1)]
                    nc.vector.tensor_scalar(
                        d2.rearrange("p (t h) -> p t h", h=H + 1)[:, :, 0:H],
                        vt4v, misc_sb[:, 4 * lc:4 * lc + 1], None,
                        mybir.AluOpType.mult)

            def kv_chunk(sc):
                kv_a(sc)
                kv_b(sc)
                kv_c(sc)

            def z_exp(s_, p, e_ap, name):
                """z matmul pair + exp for slot s_, tile pair p -> e_ap."""
                j0, j1 = 2 * p, 2 * p + 1
                z = zp.tile([128, 1024], F32, tag="z", name=f"z{name}")
                # two K=64 matmuls in different PE row groups -> concurrent
                nc.tensor.matmul(z[:, 0:512],
                                 kT2[0:64, j0 * 128:(j0 + 1) * 128],
                                 qT2[0:64, s_ * QW:(s_ + 1) * QW],
                                 start=True, stop=True)
                nc.tensor.matmul(z[:, 512:1024],
                                 kT2[64:128, j1 * 128:(j1 + 1) * 128],
                                 qT2[64:128, s_ * QW:(s_ + 1) * QW],
                                 start=True, stop=True)
                nc.scalar.activation(e_ap, z[:],
                                     mybir.ActivationFunctionType.Exp,
                                     scale=0.125)

            def pre3(p):
                z_exp(3, p, eq3[:, p * 1024:(p + 1) * 1024], f"pre3_{p}")

            def attn_slot(s_, inject=(), order=None, npre=0):
                ts_ = T[s_]
                np_ = ts_ // 2   # tile pairs
                inj = dict(inject)
                diag_jg = DIAG_JG[s_]
                oob_jg = OOB_JG[s_]
                ops = op_.tile([H + 1, 512], F32, tag="oacc", name=f"oacc{s_}")
                if order is None:
                    order = list(range(np_))
                for i, p in enumerate(order):
                    for th in inj.pop(i, ()):
                        th()
                    j0, j1 = 2 * p, 2 * p + 1
                    jg, w = p // 2, p % 2
                    if p < npre:
                        e_ap = eq3[:, p * 1024:(p + 1) * 1024]
                    else:
                        e = ep.tile([128, 1024], BF, tag="e", name=f"e{s_}_{p}")
                        e_ap = e[:]
                        z_exp(s_, p, e_ap, f"{s_}_{p}")
                        if jg == diag_jg:
                            # causal mask for the diagonal 512-block: keep
                            # where q >= p + 128t (chunk-local), else 0
                            nc.gpsimd.affine_select(
                                out=e_ap, in_=e_ap,
                                compare_op=mybir.AluOpType.is_ge,
                                fill=0.0, base=-256 * w,
                                channel_multiplier=-1,
                                pattern=[[-128, 2], [1, 512]])
                    if jg == oob_jg:
                        t0 = 4 * OOB_LC[oob_jg] + 2 * (p - 2 * oob_jg)
                        v0 = vws2[:, t0 * (H + 1):(t0 + 1) * (H + 1)]
                        v1 = vws2[:, (t0 + 1) * (H + 1):(t0 + 2) * (H + 1)]
                    else:
                        v0 = vws[:, j0 * (H + 1):(j0 + 1) * (H + 1)]
                        v1 = vws[:, j1 * (H + 1):(j1 + 1) * (H + 1)]
                    nc.tensor.matmul(ops[:], v0, e_ap[:, 0:512],
                                     start=(i == 0), stop=False)
                    nc.tensor.matmul(ops[:], v1, e_ap[:, 512:1024],
                                     start=False, stop=(i == np_ - 1))
                osb = osp.tile([H + 1, 512], BF, tag="osb", name=f"osb{s_}")
                # keep ACT free for exp; DVE has slack during attention
                nc.vector.tensor_copy(osb[:], ops[:])
                nc.sync.dma_start(o[s_], osb[:])

            # wave 0
            kv_chunk(0)
            qproj(0)
            # chunk-0 pairs of attn0 start immediately; every later
            # kv/qproj half-chain is injected one-stage-per-pair so no chain
            # exceeds the buffered-exp coverage
            attn_slot(0, inject={
                1: (lambda: kv_a(1), lambda: kv_b(1)),
                2: (lambda: kv_c(1), lambda: qproj_a(1)),
                3: (lambda: qproj_b(1),),
            })
            attn_slot(1, inject={
                1: (lambda: kv_a(2),),
                2: (lambda: kv_b(2),),
                3: (lambda: kv_c(2),),
                4: (lambda: kv_a(3),),
                5: (lambda: kv_b(3),),
                6: (lambda: kv_c(3),),
                7: (lambda: qproj_a(2),),
            })
            attn_slot(2, inject={
                0: (lambda: qproj_b(2),),
                1: (lambda: qproj_a(3),),
                2: (lambda: qproj_b(3),),
                3: (lambda: kv_a(4),),
                4: (lambda: kv_b(4),),
                5: (lambda: kv_c(4),),
                6: (lambda: kv_a(5),),
                7: (lambda: kv_b(5),),
                8: (lambda: kv_c(5), lambda: pre3(0)),
                9: (lambda: pre3(1),),
                10: (lambda: pre3(2),),
                11: (lambda: pre3(3),),
            })
            attn_slot(3, npre=4,
                      order=list(range(8, 16)) + list(range(4, 8))
                      + list(range(4)),
                      inject={
                0: (lambda: kv_a(6),),
                1: (lambda: kv_b(6),),
                2: (lambda: kv_c(6),),
                4: (lambda: kv_a(7),),
                5: (lambda: kv_b(7),),
                6: (lambda: kv_c(7),),
            })

    nc.compile()
    return nc


def _get_nc():
    if "nc" not in _cache:
        _cache["nc"] = _build()
    return _cache["nc"]


def kernel(x, Wk, Wq, Wv):
    x = np.asarray(x, dtype=np.float32)
    Wk = np.asarray(Wk, dtype=np.float32)
    Wq = np.asarray(Wq, dtype=np.float32)
    Wv = np.asarray(Wv, dtype=np.float32)

    nc = _get_nc()

    # wall[p, d*256 + 0:128]   = Wq[d*128+p, :] | Wq  (duplicated)
    # wall[p, d*256 + 128:256] = Wk[d*128+p, 0:64] | Wv
    wqq = np.concatenate([Wq, Wq], axis=1)        # [1024, 128]
    wkv = np.concatenate([Wk, Wv], axis=1)        # [1024, 128]
    wboth = np.concatenate([wqq, wkv], axis=1)    # [1024, 256]
    wall_np = np.ascontiguousarray(
        wboth.reshape(ND, 128, 256).transpose(1, 0, 2).reshape(128, ND * 256)
    ).astype(ml_dtypes.bfloat16)

    xt_b = [np.ascontiguousarray(x[b].T).astype(ml_dtypes.bfloat16)
            for b in range(B)]
    # [D, S] -> [128, ND, S] partition-major swizzle
    xt3_b = [np.ascontiguousarray(xb.reshape(ND, 128, S).transpose(1, 0, 2))
             for xb in xt_b]

    # per-class k-chunk permutation: places each class's q-chunks at the
    # uniform positions (0,3,4,7) while preserving causal prefix coverage
    PERM = [(0, 1, 2, 3, 4, 5, 6, 7), (1, 0, 3, 2, 5, 4, 7, 6)]

    in_maps = []
    for c in range(8):
        b, cls = c >> 1, c & 1
        pos = POS[cls]
        perm = PERM[cls]
        xt3_np = np.concatenate(
            [xt3_b[b][:, :, pc * 512:(pc + 1) * 512] for pc in perm], axis=2)
        qx_np = np.concatenate(
            [xt3_b[b][:, :, pos[w] * QW:(pos[w] + 1) * QW] for w in (1, 2, 3)],
            axis=2)
        # validity of the maybe-OOB groups: for slot s the group OOB_JG[s]
        # is valid iff its original chunk index <= pos[s]
        misc_np = np.zeros((128, 16), np.float32)
        for s_ in range(NQ):
            jg = OOB_JG[s_]
            lc = OOB_LC[jg]
            valid = 1.0 if perm[jg] <= pos[s_] else 0.0
            misc_np[:, 4 * lc:4 * lc + 4] = valid
        in_maps.append({
            "xt3": np.ascontiguousarray(xt3_np),
            "qx": np.ascontiguousarray(qx_np),
            "wall": wall_np,
            "misc": misc_np,
        })

    trace = bool(int(os.environ.get("KERNEL_TRACE", "0")))
    res = run_bass_kernel_spmd(nc, in_maps, core_ids=list(range(8)), trace=trace)
    _cache["last_result"] = res

    out = np.zeros((B, S, H), np.float32)
    for c in range(8):
        b, cls = c >> 1, c & 1
        oc = res.results[c]["o"].astype(np.float32)   # [NQ, 65, 512] bf16
        for s_, p in enumerate(POS[cls]):
            num = oc[s_, 0:H, :]          # [64, 512]
            den = oc[s_, H, :]            # [512]
            out[b, p * QW:(p + 1) * QW, :] = (num / den[None, :]).T
    return out


# revision 8
# speedup vs baseline: 1.0440x; 1.0440x over previous
import sys

sys.path.insert(0, "/opt/trn_rl_repo")

import os
import numpy as np
import ml_dtypes

import concourse.bass as bass
import concourse.mybir as mybir
import concourse.tile as tile
from concourse import bacc
from concourse.bass_utils import run_bass_kernel_spmd
from concourse.masks import make_identity

B, S, D, H = 4, 4096, 1024, 64
QW = 512                      # q-chunk width
NQ = 4                        # q-chunk slots per core
POS = [(0, 3, 4, 7), (1, 2, 5, 6)]   # q-chunk positions per core class
T = (8, 16, 24, 32)           # k-tiles (128 wide) per slot in the uniform graph
NKT = S // 128                # 32 k tiles
NC = 8                        # permuted 512-chunks
ND = D // 128                 # 8 d-tiles
DIAG_JG = (0, 3, 4, 7)        # diagonal 512-group per slot (class-independent)
OOB_JG = (1, 2, 5, 6)         # maybe-out-of-bounds group per slot
OOB_LC = {1: 0, 2: 1, 5: 2, 6: 3}   # chunk -> local index in vws2

BF = mybir.dt.bfloat16
F32 = mybir.dt.float32

_cache = {}


def _build():
    nc = bacc.Bacc("TRN2", target_bir_lowering=False, debug=False, num_devices=8)

    # host pre-swizzled inputs: chunk-contiguous so each chunk DMA is 128
    # descriptors of 8KB (descriptor GEN on the sync sequencer paces the
    # input stream, not queue bandwidth)
    xt3 = nc.dram_tensor("xt3", [128, NC, ND * 512], BF, kind="ExternalInput").ap()
    # slot-1/2/3 q-chunk blocks, loaded early so no qproj waits on the
    # main chunk stream
    qx = nc.dram_tensor("qx", [128, 3, ND * 512], BF, kind="ExternalInput").ap()
    wall = nc.dram_tensor("wall", [128, ND * 256], BF, kind="ExternalInput").ap()
    # per-tile validity scalars for the maybe-OOB groups (chunks 1,2,5,6)
    misc = nc.dram_tensor("misc", [128, 16], F32, kind="ExternalInput").ap()
    o = nc.dram_tensor("o", [NQ, H + 1, QW], BF, kind="ExternalOutput").ap()

    with tile.TileContext(nc) as tc:
        with (
            tc.tile_pool(name="persist", bufs=1) as pp,
            tc.tile_pool(name="xin", bufs=1) as xp,
            tc.tile_pool(name="estage", bufs=6) as ep,
            tc.tile_pool(name="vstage", bufs=3) as vsp,
            tc.tile_pool(name="ostage", bufs=2) as osp,
            tc.tile_pool(name="zpsum", bufs=2, space="PSUM") as zp,
            tc.tile_pool(name="opsum", bufs=1, space="PSUM") as op_,
            tc.tile_pool(name="projpsum", bufs=2, space="PSUM") as prp,
            tc.tile_pool(name="vtpsum", bufs=1, space="PSUM") as vtp,
        ):
            # ---- persistent tiles ----
            wall_sb = pp.tile([128, ND * 256], BF, tag="wall")
            misc_sb = pp.tile([128, 16], F32, tag="misc")
            ident = pp.tile([64, 64], BF, tag="ident")
            qT2 = pp.tile([128, NQ * QW], BF, tag="qT2")
            kT2 = pp.tile([128, S], BF, tag="kT2")
            vws = pp.tile([128, NKT * (H + 1)], BF, tag="vws")
            # zero-or-copy of v for the maybe-OOB groups (chunks 1,2,5,6)
            vws2 = pp.tile([128, 16 * (H + 1)], BF, tag="vws2")
            # e-queue: slot-3 pairs 0-7 exp'd early inside attn2's holes
            eq3 = pp.tile([128, 8 * 1024], BF, tag="eq3")
            xtall = xp.tile([128, NC, ND, 512], BF, tag="xtall")
            qxall = xp.tile([128, 3, ND, 512], BF, tag="qxall")

            # ---- input DMAs: one trigger per logical block, consumption order
            nc.sync.dma_start(misc_sb[:], misc[:])
            # first chunk split by d-halves so the kv accumulation can
            # start before the full chunk lands
            x3v = xtall[:].rearrange("p c d w -> p c (d w)")
            nc.sync.dma_start(x3v[:, 0:1, 0:2048], xt3[:, 0:1, 0:2048])
            nc.sync.dma_start(x3v[:, 0:1, 2048:4096], xt3[:, 0:1, 2048:4096])
            nc.sync.dma_start(wall_sb[:], wall[:])
            qxv = qxall[:].rearrange("p c d w -> p c (d w)")
            nc.sync.dma_start(qxv[:, 0:1], qx[:, 0:1])
            nc.sync.dma_start(x3v[:, 1:2], xt3[:, 1:2])
            nc.sync.dma_start(qxv[:, 1:3], qx[:, 1:3])
            for c in range(2, NC):
                nc.sync.dma_start(x3v[:, c:c + 1], xt3[:, c:c + 1])

            make_identity(nc, ident[:])
            # hold the PE busy (HAM warm) on the identity tile (available
            # immediately, no DMA dependency) while inputs stream in
            wtile = vtp.tile([128, 64], F32, tag="vt", name="warmps")
            for i in range(48):
                nc.tensor.matmul(wtile[0:64, :], ident[:], ident[:],
                                 start=True, stop=True)
            nc.gpsimd.memset(vws[:], 1.0)
            # vws2 ones-row = per-tile validity (0/1 from host)
            nc.gpsimd.memset(vws2[:], 0.0)
            v2ones = vws2[:].rearrange("p (t h) -> p t h", h=H + 1)[:, :, H:H + 1]
            nc.vector.tensor_copy(v2ones, misc_sb[:].rearrange(
                "p (t u) -> p t u", u=1))
            # warm the ACT exp table early
            warm = ep.tile([128, 1], BF, tag="warm")
            nc.scalar.activation(warm[:], misc_sb[:, 0:1],
                                 mybir.ActivationFunctionType.Exp)

            _qps = {}

            def qproj_a(w):
                ps = prp.tile([128, 512], F32, tag="proj", name=f"qps{w}")
                _qps[w] = ps
                for d in range(4):
                    if w >= 1:
                        rhs = qxall[:, w - 1, d, :]
                    else:
                        rhs = xtall[:, 0, d, :]
                    nc.tensor.matmul(ps[:], wall_sb[:, d * 256:d * 256 + 128],
                                     rhs, start=(d == 0), stop=False)

            def qproj_b(w):
                ps = _qps.pop(w)
                for d in range(4, ND):
                    if w >= 1:
                        rhs = qxall[:, w - 1, d, :]
                    else:
                        rhs = xtall[:, 0, d, :]
                    nc.tensor.matmul(ps[:], wall_sb[:, d * 256:d * 256 + 128],
                                     rhs, start=False, stop=(d == ND - 1))
                # rows 0-63 and 64-127 both hold q^T (duplicated weights)
                nc.vector.tensor_copy(qT2[:, w * QW:(w + 1) * QW], ps[:])

            def qproj(w):
                qproj_a(w)
                qproj_b(w)

            _kvps = {}
            _kvvst = {}

            def kv_a(sc):
                ps = prp.tile([128, 512], F32, tag="proj", name=f"kvps{sc}")
                _kvps[sc] = ps
                for d in range(4):
                    nc.tensor.matmul(ps[:], wall_sb[:, d * 256 + 128:d * 256 + 256],
                                     xtall[:, sc, d, :],
                                     start=(d == 0), stop=False)

            def kv_b(sc):
                ps = _kvps.pop(sc)
                for d in range(4, ND):
                    nc.tensor.matmul(ps[:], wall_sb[:, d * 256 + 128:d * 256 + 256],
                                     xtall[:, sc, d, :],
                                     start=False, stop=(d == ND - 1))
                nc.vector.tensor_copy(kT2[0:64, sc * 512:(sc + 1) * 512], ps[0:64, :])
                # duplicate k^T into the upper partition half (for row-tiled QK)
                # via an identity matmul into PE col-group (0,64) — no DMA.
                # kdup shares the vt pool so the z pool stays free for the
                # 2-deep z pipeline
                kdup = vtp.tile([128, 512], F32, tag="vt", name=f"kdup{sc}")
                nc.tensor.matmul(kdup[64:128, :], ident[:],
                                 kT2[0:64, sc * 512:(sc + 1) * 512],
                                 start=True, stop=True)
                nc.vector.tensor_copy(kT2[64:128, sc * 512:(sc + 1) * 512],
                                      kdup[64:128, :])
                vstage = vsp.tile([64, 512], BF, tag="vstage", name=f"vst{sc}")
                nc.vector.tensor_copy(vstage[:], ps[64:128, :])
                _kvvst[sc] = vstage

            def kv_c(sc):
                vstage = _kvvst.pop(sc)
                # all 4 transposes into one PSUM tile (68-col stride keeps the
                # matmul writes 8B-aligned), then ONE strided copy into vws
                vt4 = vtp.tile([128, 4 * 68], BF, tag="vt", name=f"vt4_{sc}")
                for t in range(4):
                    nc.tensor.transpose(vt4[:, t * 68:t * 68 + 64],
                                        vstage[:, t * 128:(t + 1) * 128],
                                        ident[:])
                kt0 = 4 * sc
                dst = vws[:, kt0 * (H + 1):(kt0 + 4) * (H + 1)]
                vt4v = vt4[:].rearrange("p (t h) -> p t h", h=68)[:, :, 0:H]
                nc.vector.tensor_copy(
                    dst.rearrange("p (t h) -> p t h", h=H + 1)[:, :, 0:H], vt4v)
                if sc in OOB_LC:
                    # scaled copy into vws2 (valid -> v, invalid -> 0)
                    lc = OOB_LC[sc]
                    d2 = vws2[:, lc * 4 * (H + 1):(lc + 1) * 4 * (H + 1)]
                    nc.vector.tensor_scalar(
                        d2.rearrange("p (t h) -> p t h", h=H + 1)[:, :, 0:H],
                        vt4v, misc_sb[:, 4 * lc:4 * lc + 1], None,
                        mybir.AluOpType.mult)

            def kv_chunk(sc):
                kv_a(sc)
                kv_b(sc)
                kv_c(sc)

            def z_exp(s_, p, e_ap, name):
                """z matmul pair + exp (+ causal mask) for slot s_, pair p."""
                j0, j1 = 2 * p, 2 * p + 1
                z = zp.tile([128, 1024], F32, tag="z", name=f"z{name}")
                # two K=64 matmuls in different PE row groups -> concurrent
                nc.tensor.matmul(z[:, 0:512],
                                 kT2[0:64, j0 * 128:(j0 + 1) * 128],
                                 qT2[0:64, s_ * QW:(s_ + 1) * QW],
                                 start=True, stop=True)
                nc.tensor.matmul(z[:, 512:1024],
                                 kT2[64:128, j1 * 128:(j1 + 1) * 128],
                                 qT2[64:128, s_ * QW:(s_ + 1) * QW],
                                 start=True, stop=True)
                nc.scalar.activation(e_ap, z[:],
                                     mybir.ActivationFunctionType.Exp,
                                     scale=0.125)
                if p // 2 == DIAG_JG[s_]:
                    # causal mask for the diagonal 512-block: keep where
                    # q >= p + 128t (chunk-local), else 0
                    nc.gpsimd.affine_select(
                        out=e_ap, in_=e_ap,
                        compare_op=mybir.AluOpType.is_ge,
                        fill=0.0, base=-256 * (p % 2),
                        channel_multiplier=-1,
                        pattern=[[-128, 2], [1, 512]])

            def pre3(p):
                z_exp(3, p, eq3[:, p * 1024:(p + 1) * 1024], f"pre3_{p}")

            def attn_slot(s_, inject=(), order=None, npre=0, pre_inject=()):
                ts_ = T[s_]
                np_ = ts_ // 2   # tile pairs
                inj = dict(inject)
                oob_jg = OOB_JG[s_]
                for th in pre_inject:
                    th()
                ops = op_.tile([H + 1, 512], F32, tag="oacc", name=f"oacc{s_}")
                if order is None:
                    order = list(range(np_))
                _es = {}

                def emit_z(p):
                    if p < npre:
                        _es[p] = eq3[:, p * 1024:(p + 1) * 1024]
                    else:
                        e = ep.tile([128, 1024], BF, tag="e", name=f"e{s_}_{p}")
                        _es[p] = e[:]
                        z_exp(s_, p, e[:], f"{s_}_{p}")

                # software pipeline: keep 2 z/exp stages in flight ahead of PV
                emit_z(order[0])
                if np_ > 1:
                    emit_z(order[1])
                for i, p in enumerate(order):
                    for th in inj.pop(i, ()):
                        th()
                    if i + 2 < np_:
                        emit_z(order[i + 2])
                    e_ap = _es.pop(p)
                    j0, j1 = 2 * p, 2 * p + 1
                    jg = p // 2
                    if jg == oob_jg:
                        t0 = 4 * OOB_LC[oob_jg] + 2 * (p - 2 * oob_jg)
                        v0 = vws2[:, t0 * (H + 1):(t0 + 1) * (H + 1)]
                        v1 = vws2[:, (t0 + 1) * (H + 1):(t0 + 2) * (H + 1)]
                    else:
                        v0 = vws[:, j0 * (H + 1):(j0 + 1) * (H + 1)]
                        v1 = vws[:, j1 * (H + 1):(j1 + 1) * (H + 1)]
                    nc.tensor.matmul(ops[:], v0, e_ap[:, 0:512],
                                     start=(i == 0), stop=False)
                    nc.tensor.matmul(ops[:], v1, e_ap[:, 512:1024],
                                     start=False, stop=(i == np_ - 1))
                osb = osp.tile([H + 1, 512], BF, tag="osb", name=f"osb{s_}")
                # keep ACT free for exp; DVE has slack during attention
                nc.vector.tensor_copy(osb[:], ops[:])
                nc.sync.dma_start(o[s_], osb[:])

            # wave 0
            kv_chunk(0)
            qproj(0)
            # chunk-0 pairs of attn0 start immediately; every later
            # kv/qproj half-chain is injected one-stage-per-pair so no chain
            # exceeds the buffered-exp coverage
            attn_slot(0, inject={
                0: (lambda: kv_a(1), lambda: kv_b(1)),
                1: (lambda: kv_c(1), lambda: qproj_a(1)),
                2: (lambda: qproj_b(1),),
            })
            attn_slot(1, inject={
                1: (lambda: kv_a(2),),
                2: (lambda: kv_b(2),),
                3: (lambda: kv_c(2), lambda: kv_a(3)),
                4: (lambda: kv_b(3),),
                5: (lambda: kv_c(3),),
                6: (lambda: qproj_a(2),),
            })
            attn_slot(2, pre_inject=(lambda: qproj_b(2),), inject={
                1: (lambda: qproj_a(3),),
                2: (lambda: qproj_b(3),),
                3: (lambda: kv_a(4), lambda: pre3(0)),
                4: (lambda: kv_b(4), lambda: pre3(1)),
                5: (lambda: kv_c(4), lambda: pre3(2)),
                6: (lambda: kv_a(5), lambda: pre3(3)),
                7: (lambda: kv_b(5), lambda: pre3(4)),
                8: (lambda: kv_c(5), lambda: pre3(5)),
                9: (lambda: pre3(6),),
                10: (lambda: pre3(7),),
            })
            attn_slot(3, npre=8,
                      order=list(range(8, 16)) + list(range(4, 8))
                      + list(range(4)),
                      inject={
                0: (lambda: kv_a(6),),
                1: (lambda: kv_b(6),),
                2: (lambda: kv_c(6),),
                3: (lambda: kv_a(7),),
                4: (lambda: kv_b(7),),
                5: (lambda: kv_c(7),),
            })

    nc.compile()
    return nc


def _get_nc():
    if "nc" not in _cache:
        _cache["nc"] = _build()
    return _cache["nc"]


def kernel(x, Wk, Wq, Wv):
    x = np.asarray(x, dtype=np.float32)
    Wk = np.asarray(Wk, dtype=np.float32)
    Wq = np.asarray(Wq, dtype=np.float32)
    Wv = np.asarray(Wv, dtype=np.float32)

    nc = _get_nc()

    # wall[p, d*256 + 0:128]   = Wq[d*128+p, :] | Wq  (duplicated)
    # wall[p, d*256 + 128:256] = Wk[d*128+p, 0:64] | Wv
    wqq = np.concatenate([Wq, Wq], axis=1)        # [1024, 128]
    wkv = np.concatenate([Wk, Wv], axis=1)        # [1024, 128]
    wboth = np.concatenate([wqq, wkv], axis=1)    # [1024, 256]
    wall_np = np.ascontiguousarray(
        wboth.reshape(ND, 128, 256).transpose(1, 0, 2).reshape(128, ND * 256)
    ).astype(ml_dtypes.bfloat16)

    xt_b = [np.ascontiguousarray(x[b].T).astype(ml_dtypes.bfloat16)
            for b in range(B)]
    # [D, S] -> [128, ND, S] partition-major swizzle
    xt3_b = [np.ascontiguousarray(xb.reshape(ND, 128, S).transpose(1, 0, 2))
             for xb in xt_b]

    # per-class k-chunk permutation: places each class's q-chunks at the
    # uniform positions (0,3,4,7) while preserving causal prefix coverage
    PERM = [(0, 1, 2, 3, 4, 5, 6, 7), (1, 0, 3, 2, 5, 4, 7, 6)]

    in_maps = []
    for c in range(8):
        b, cls = c >> 1, c & 1
        pos = POS[cls]
        perm = PERM[cls]
        # chunk-contiguous: [128, NC, ND*512]
        xt3_np = np.stack(
            [xt3_b[b][:, :, pc * 512:(pc + 1) * 512].reshape(128, ND * 512)
             for pc in perm], axis=1)
        qx_np = np.stack(
            [xt3_b[b][:, :, pos[w] * QW:(pos[w] + 1) * QW].reshape(128, ND * 512)
             for w in (1, 2, 3)], axis=1)
        # validity of the maybe-OOB groups: for slot s the group OOB_JG[s]
        # is valid iff its original chunk index <= pos[s]
        misc_np = np.zeros((128, 16), np.float32)
        for s_ in range(NQ):
            jg = OOB_JG[s_]
            lc = OOB_LC[jg]
            valid = 1.0 if perm[jg] <= pos[s_] else 0.0
            misc_np[:, 4 * lc:4 * lc + 4] = valid
        in_maps.append({
            "xt3": np.ascontiguousarray(xt3_np),
            "qx": np.ascontiguousarray(qx_np),
            "wall": wall_np,
            "misc": misc_np,
        })

    trace = bool(int(os.environ.get("KERNEL_TRACE", "0")))
    res = run_bass_kernel_spmd(nc, in_maps, core_ids=list(range(8)), trace=trace)
    _cache["last_result"] = res

    out = np.zeros((B, S, H), np.float32)
    for c in range(8):
        b, cls = c >> 1, c & 1
        oc = res.results[c]["o"].astype(np.float32)   # [NQ, 65, 512] bf16
        for s_, p in enumerate(POS[cls]):
            num = oc[s_, 0:H, :]          # [64, 512]
            den = oc[s_, H, :]            # [512]
            out[b, p * QW:(p + 1) * QW, :] = (num / den[None, :]).T
    return out


# revision 10
# speedup vs baseline: 1.0681x; 1.0231x over previous
import sys

sys.path.insert(0, "/opt/trn_rl_repo")

import os
import numpy as np
import ml_dtypes

import concourse.bass as bass
import concourse.mybir as mybir
import concourse.tile as tile
from concourse import bacc
from concourse.bass_utils import run_bass_kernel_spmd
from concourse.masks import make_identity

B, S, D, H = 4, 4096, 1024, 64
QW = 512                      # q-chunk width
NQ = 4                        # q-chunk slots per core
POS = [(0, 3, 4, 7), (1, 2, 5, 6)]   # q-chunk positions per core class
T = (8, 16, 24, 32)           # k-tiles (128 wide) per slot in the uniform graph
NKT = S // 128                # 32 k tiles
NC = 8                        # permuted 512-chunks
ND = D // 128                 # 8 d-tiles
DIAG_JG = (0, 3, 4, 7)        # diagonal 512-group per slot (class-independent)
OOB_JG = (1, 2, 5, 6)         # maybe-out-of-bounds group per slot
OOB_LC = {1: 0, 2: 1, 5: 2, 6: 3}   # chunk -> local index in vws2

BF = mybir.dt.bfloat16
F32 = mybir.dt.float32

_cache = {}


def _build():
    nc = bacc.Bacc("TRN2", target_bir_lowering=False, debug=False, num_devices=8)

    # host pre-swizzled inputs: chunk-contiguous so each chunk DMA is 128
    # descriptors of 8KB (descriptor GEN on the sync sequencer paces the
    # input stream, not queue bandwidth)
    xt3 = nc.dram_tensor("xt3", [128, NC, ND * 512], BF, kind="ExternalInput").ap()
    # slot-1/2/3 q-chunk blocks, loaded early so no qproj waits on the
    # main chunk stream
    qx = nc.dram_tensor("qx", [128, 3, ND * 512], BF, kind="ExternalInput").ap()
    wall = nc.dram_tensor("wall", [128, ND * 384], BF, kind="ExternalInput").ap()
    # per-tile validity scalars for the maybe-OOB groups (chunks 1,2,5,6)
    misc = nc.dram_tensor("misc", [128, 16], F32, kind="ExternalInput").ap()
    o = nc.dram_tensor("o", [NQ, H + 1, QW], BF, kind="ExternalOutput").ap()

    with tile.TileContext(nc) as tc:
        with (
            tc.tile_pool(name="persist", bufs=1) as pp,
            tc.tile_pool(name="xin", bufs=1) as xp,
            tc.tile_pool(name="estage", bufs=6) as ep,
            tc.tile_pool(name="vstage", bufs=3) as vsp,
            tc.tile_pool(name="ostage", bufs=2) as osp,
            tc.tile_pool(name="zpsum", bufs=2, space="PSUM") as zp,
            tc.tile_pool(name="opsum", bufs=1, space="PSUM") as op_,
            tc.tile_pool(name="projpsum", bufs=2, space="PSUM") as prp,
            tc.tile_pool(name="vtpsum", bufs=1, space="PSUM") as vtp,
        ):
            # ---- persistent tiles ----
            wall_sb = pp.tile([128, ND * 384], BF, tag="wall")
            misc_sb = pp.tile([128, 16], F32, tag="misc")
            ident = pp.tile([64, 64], BF, tag="ident")
            qT2 = pp.tile([128, NQ * QW], BF, tag="qT2")
            kT2 = pp.tile([128, S], BF, tag="kT2")
            vws = pp.tile([128, NKT * (H + 1)], BF, tag="vws")
            # zero-or-copy of v for the maybe-OOB groups (chunks 1,2,5,6)
            vws2 = pp.tile([128, 16 * (H + 1)], BF, tag="vws2")
            # e-queue: slot-3 pairs 0-7 exp'd early inside attn2's holes
            eq3 = pp.tile([128, 8 * 1024], BF, tag="eq3")
            xtall = xp.tile([128, NC, ND, 512], BF, tag="xtall")
            qxall = xp.tile([128, 3, ND, 512], BF, tag="qxall")

            # ---- input DMAs: one trigger per logical block, consumption order
            nc.sync.dma_start(misc_sb[:], misc[:])
            nc.sync.dma_start(wall_sb[:], wall[:])
            # first chunk split by d-halves so the kv accumulation can
            # start before the full chunk lands
            x3v = xtall[:].rearrange("p c d w -> p c (d w)")
            nc.sync.dma_start(x3v[:, 0:1, 0:2048], xt3[:, 0:1, 0:2048])
            nc.sync.dma_start(x3v[:, 0:1, 2048:4096], xt3[:, 0:1, 2048:4096])
            qxv = qxall[:].rearrange("p c d w -> p c (d w)")
            nc.sync.dma_start(qxv[:, 0:1], qx[:, 0:1])
            nc.sync.dma_start(x3v[:, 1:2], xt3[:, 1:2])
            nc.sync.dma_start(qxv[:, 1:3], qx[:, 1:3])
            for c in range(2, NC):
                nc.sync.dma_start(x3v[:, c:c + 1], xt3[:, c:c + 1])

            make_identity(nc, ident[:])
            # hold the PE busy (HAM warm) on the identity tile (available
            # immediately, no DMA dependency) while inputs stream in
            wtile = vtp.tile([128, 64], F32, tag="vt", name="warmps")
            for i in range(60):
                nc.tensor.matmul(wtile[0:64, :], ident[:], ident[:],
                                 start=True, stop=True)
            # big memsets on DVE so the gpsimd queue stays clear for the
            # identity build (gpsimd would delay the PE warmup otherwise)
            nc.vector.memset(vws[:], 1.0)
            nc.vector.memset(vws2[:], 0.0)
            # vws2 ones-row = per-tile validity (0/1 from host)
            v2ones = vws2[:].rearrange("p (t h) -> p t h", h=H + 1)[:, :, H:H + 1]
            nc.vector.tensor_copy(v2ones, misc_sb[:].rearrange(
                "p (t u) -> p t u", u=1))
            # warm the ACT exp table early
            warm = ep.tile([128, 1], BF, tag="warm")
            nc.scalar.activation(warm[:], misc_sb[:, 0:1],
                                 mybir.ActivationFunctionType.Exp)

            _qps = {}

            def qproj_a(w):
                ps = prp.tile([128, 512], F32, tag="proj", name=f"qps{w}")
                _qps[w] = ps
                for d in range(4):
                    if w >= 1:
                        rhs = qxall[:, w - 1, d, :]
                    else:
                        rhs = xtall[:, 0, d, :]
                    nc.tensor.matmul(ps[:], wall_sb[:, d * 384:d * 384 + 128],
                                     rhs, start=(d == 0), stop=False)

            def qproj_b(w):
                ps = _qps.pop(w)
                for d in range(4, ND):
                    if w >= 1:
                        rhs = qxall[:, w - 1, d, :]
                    else:
                        rhs = xtall[:, 0, d, :]
                    nc.tensor.matmul(ps[:], wall_sb[:, d * 384:d * 384 + 128],
                                     rhs, start=False, stop=(d == ND - 1))
                # rows 0-63 and 64-127 both hold q^T (duplicated weights)
                nc.vector.tensor_copy(qT2[:, w * QW:(w + 1) * QW], ps[:])

            def qproj(w):
                qproj_a(w)
                qproj_b(w)

            _kvps = {}
            _kvvst = {}

            def _xev(sc, d, half):
                # even (half=0) or odd (half=1) 128-tiles of chunk sc, d-tile d
                return xtall[:, sc, d, :].rearrange(
                    "p (a b w) -> p a b w", a=2, b=2)[:, :, half, :]

            def kv_1(sc):
                # A-half: [wk|wv] weights on the EVEN k-tiles of the chunk
                ps = prp.tile([128, 512], F32, tag="proj", name=f"kvps{sc}")
                _kvps[sc] = ps
                for d in range(ND):
                    nc.tensor.matmul(ps[:, 0:256],
                                     wall_sb[:, d * 384 + 128:d * 384 + 256],
                                     _xev(sc, d, 0),
                                     start=(d == 0), stop=(d == ND - 1))

            def kv_2(sc):
                # B-half: [wv|wk] weights on the ODD k-tiles -> k lands in
                # partitions 64:128 directly (no kdup matmul needed)
                ps = _kvps[sc]
                for d in range(ND):
                    nc.tensor.matmul(ps[:, 256:512],
                                     wall_sb[:, d * 384 + 256:d * 384 + 384],
                                     _xev(sc, d, 1),
                                     start=(d == 0), stop=(d == ND - 1))
                # A-half copies: k of even tiles -> kT2 rows 0:64
                kdst = kT2[0:64, sc * 512:(sc + 1) * 512].rearrange(
                    "p (a b w) -> p a b w", a=2, b=2)
                nc.vector.tensor_copy(
                    kdst[:, :, 0, :],
                    ps[0:64, 0:256].rearrange("p (t w) -> p t w", w=128))
                vstage = vsp.tile([64, 512], BF, tag="vstage", name=f"vst{sc}")
                _kvvst[sc] = vstage
                vdst = vstage[:].rearrange("p (a b w) -> p a b w", a=2, b=2)
                nc.vector.tensor_copy(
                    vdst[:, :, 0, :],
                    ps[64:128, 0:256].rearrange("p (t w) -> p t w", w=128))

            def kv_3(sc):
                # B-half copies: k of odd tiles -> kT2 rows 64:128
                ps = _kvps.pop(sc)
                kdst = kT2[64:128, sc * 512:(sc + 1) * 512].rearrange(
                    "p (a b w) -> p a b w", a=2, b=2)
                nc.vector.tensor_copy(
                    kdst[:, :, 1, :],
                    ps[64:128, 256:512].rearrange("p (t w) -> p t w", w=128))
                vstage = _kvvst[sc]
                vdst = vstage[:].rearrange("p (a b w) -> p a b w", a=2, b=2)
                nc.vector.tensor_copy(
                    vdst[:, :, 1, :],
                    ps[0:64, 256:512].rearrange("p (t w) -> p t w", w=128))

            def kv_4(sc):
                vstage = _kvvst.pop(sc)
                # all 4 transposes into one PSUM tile (68-col stride keeps the
                # matmul writes 8B-aligned), then ONE strided copy into vws
                vt4 = vtp.tile([128, 4 * 68], BF, tag="vt", name=f"vt4_{sc}")
                for t in range(4):
                    nc.tensor.transpose(vt4[:, t * 68:t * 68 + 64],
                                        vstage[:, t * 128:(t + 1) * 128],
                                        ident[:])
                kt0 = 4 * sc
                dst = vws[:, kt0 * (H + 1):(kt0 + 4) * (H + 1)]
                vt4v = vt4[:].rearrange("p (t h) -> p t h", h=68)[:, :, 0:H]
                nc.vector.tensor_copy(
                    dst.rearrange("p (t h) -> p t h", h=H + 1)[:, :, 0:H], vt4v)
                if sc in OOB_LC:
                    # scaled copy into vws2 (valid -> v, invalid -> 0)
                    lc = OOB_LC[sc]
                    d2 = vws2[:, lc * 4 * (H + 1):(lc + 1) * 4 * (H + 1)]
                    nc.vector.tensor_scalar(
                        d2.rearrange("p (t h) -> p t h", h=H + 1)[:, :, 0:H],
                        vt4v, misc_sb[:, 4 * lc:4 * lc + 1], None,
                        mybir.AluOpType.mult)

            def kv_chunk(sc):
                kv_1(sc)
                kv_2(sc)
                kv_3(sc)
                kv_4(sc)

            def z_exp(s_, p, e_ap, name):
                """z matmul pair + exp (+ causal mask) for slot s_, pair p."""
                j0, j1 = 2 * p, 2 * p + 1
                z = zp.tile([128, 1024], F32, tag="z", name=f"z{name}")
                # two K=64 matmuls in different PE row groups
                nc.tensor.matmul(z[:, 0:512],
                                 kT2[0:64, j0 * 128:(j0 + 1) * 128],
                                 qT2[0:64, s_ * QW:(s_ + 1) * QW],
                                 start=True, stop=True)
                nc.tensor.matmul(z[:, 512:1024],
                                 kT2[64:128, j1 * 128:(j1 + 1) * 128],
                                 qT2[64:128, s_ * QW:(s_ + 1) * QW],
                                 start=True, stop=True)
                nc.scalar.activation(e_ap, z[:],
                                     mybir.ActivationFunctionType.Exp,
                                     scale=0.125)
                if p // 2 == DIAG_JG[s_]:
                    # causal mask for the diagonal 512-block: keep where
                    # q >= p + 128t (chunk-local), else 0
                    nc.gpsimd.affine_select(
                        out=e_ap, in_=e_ap,
                        compare_op=mybir.AluOpType.is_ge,
                        fill=0.0, base=-256 * (p % 2),
                        channel_multiplier=-1,
                        pattern=[[-128, 2], [1, 512]])

            def pre3(p):
                z_exp(3, p, eq3[:, p * 1024:(p + 1) * 1024], f"pre3_{p}")

            def attn_slot(s_, inject=(), order=None, npre=0, pre_inject=(),
                          osb_engine=None):
                ts_ = T[s_]
                np_ = ts_ // 2   # tile pairs
                inj = dict(inject)
                oob_jg = OOB_JG[s_]
                for th in pre_inject:
                    th()
                ops = op_.tile([H + 1, 512], F32, tag="oacc", name=f"oacc{s_}")
                if order is None:
                    order = list(range(np_))
                _es = {}

                def emit_z(p):
                    if p < npre:
                        _es[p] = eq3[:, p * 1024:(p + 1) * 1024]
                    else:
                        e = ep.tile([128, 1024], BF, tag="e", name=f"e{s_}_{p}")
                        _es[p] = e[:]
                        z_exp(s_, p, e[:], f"{s_}_{p}")

                # software pipeline: keep 2 z/exp stages in flight ahead of PV
                emit_z(order[0])
                if np_ > 1:
                    emit_z(order[1])
                for i, p in enumerate(order):
                    for th in inj.pop(i, ()):
                        th()
                    if i + 2 < np_:
                        emit_z(order[i + 2])
                    e_ap = _es.pop(p)
                    j0, j1 = 2 * p, 2 * p + 1
                    jg = p // 2
                    if jg == oob_jg:
                        t0 = 4 * OOB_LC[oob_jg] + 2 * (p - 2 * oob_jg)
                        v0 = vws2[:, t0 * (H + 1):(t0 + 1) * (H + 1)]
                        v1 = vws2[:, (t0 + 1) * (H + 1):(t0 + 2) * (H + 1)]
                    else:
                        v0 = vws[:, j0 * (H + 1):(j0 + 1) * (H + 1)]
                        v1 = vws[:, j1 * (H + 1):(j1 + 1) * (H + 1)]
                    nc.tensor.matmul(ops[:], v0, e_ap[:, 0:512],
                                     start=(i == 0), stop=False)
                    nc.tensor.matmul(ops[:], v1, e_ap[:, 512:1024],
                                     start=False, stop=(i == np_ - 1))
                osb = osp.tile([H + 1, 512], BF, tag="osb", name=f"osb{s_}")
                if osb_engine == "scalar":
                    nc.scalar.copy(osb[:], ops[:])
                else:
                    nc.vector.tensor_copy(osb[:], ops[:])
                nc.sync.dma_start(o[s_], osb[:])

            # wave 0
            kv_chunk(0)
            qproj(0)
            # chunk-0 pairs of attn0 start immediately; every later
            # kv/qproj stage is injected so no chain exceeds the
            # buffered-exp coverage
            attn_slot(0, inject={
                0: (lambda: kv_1(1), lambda: kv_2(1), lambda: kv_3(1)),
                1: (lambda: kv_4(1), lambda: qproj_a(1)),
                2: (lambda: qproj_b(1),),
            })
            attn_slot(1, inject={
                0: (lambda: kv_1(2),),
                1: (lambda: kv_2(2),),
                2: (lambda: kv_3(2), lambda: kv_1(3)),
                3: (lambda: kv_4(2), lambda: kv_2(3)),
                4: (lambda: kv_3(3),),
                5: (lambda: kv_4(3),),
                6: (lambda: qproj_a(2),),
            })
            attn_slot(2, pre_inject=(lambda: qproj_b(2),), inject={
                0: (lambda: qproj_a(3),),
                1: (lambda: qproj_b(3),),
                2: (lambda: kv_1(4), lambda: pre3(0)),
                3: (lambda: kv_2(4), lambda: pre3(1)),
                4: (lambda: kv_3(4), lambda: pre3(2)),
                5: (lambda: kv_4(4), lambda: pre3(3)),
                6: (lambda: kv_1(5), lambda: pre3(4)),
                7: (lambda: kv_2(5), lambda: pre3(5)),
                8: (lambda: kv_3(5), lambda: pre3(6)),
                9: (lambda: kv_4(5), lambda: pre3(7)),
            })
            # slot 3: live pairs (8-15) interleaved with PV-only pre-exp'd
            # pairs (0-7); diag pairs 14/15 moved off the tail
            attn_slot(3, npre=8, osb_engine="scalar",
                      order=[8, 0, 9, 1, 10, 2, 11, 3,
                             14, 4, 15, 5, 12, 6, 13, 7],
                      inject={
                0: (lambda: kv_1(6),),
                1: (lambda: kv_2(6),),
                2: (lambda: kv_3(6),),
                3: (lambda: kv_4(6),),
                4: (lambda: kv_1(7),),
                5: (lambda: kv_2(7),),
                6: (lambda: kv_3(7),),
                7: (lambda: kv_4(7),),
            })

    nc.compile()
    return nc


def _get_nc():
    if "nc" not in _cache:
        _cache["nc"] = _build()
    return _cache["nc"]


def kernel(x, Wk, Wq, Wv):
    x = np.asarray(x, dtype=np.float32)
    Wk = np.asarray(Wk, dtype=np.float32)
    Wq = np.asarray(Wq, dtype=np.float32)
    Wv = np.asarray(Wv, dtype=np.float32)

    nc = _get_nc()

    # wall[p, d*384 + 0:128]   = Wq[d*128+p, :] | Wq  (duplicated)
    # wall[p, d*384 + 128:256] = Wk | Wv   (A-half, even k-tiles)
    # wall[p, d*384 + 256:384] = Wv | Wk   (B-half, odd k-tiles)
    wqq = np.concatenate([Wq, Wq], axis=1)        # [1024, 128]
    wkv = np.concatenate([Wk, Wv], axis=1)        # [1024, 128]
    wvk = np.concatenate([Wv, Wk], axis=1)        # [1024, 128]
    wboth = np.concatenate([wqq, wkv, wvk], axis=1)   # [1024, 384]
    wall_np = np.ascontiguousarray(
        wboth.reshape(ND, 128, 384).transpose(1, 0, 2).reshape(128, ND * 384)
    ).astype(ml_dtypes.bfloat16)

    xt_b = [np.ascontiguousarray(x[b].T).astype(ml_dtypes.bfloat16)
            for b in range(B)]
    # [D, S] -> [128, ND, S] partition-major swizzle
    xt3_b = [np.ascontiguousarray(xb.reshape(ND, 128, S).transpose(1, 0, 2))
             for xb in xt_b]

    # per-class k-chunk permutation: places each class's q-chunks at the
    # uniform positions (0,3,4,7) while preserving causal prefix coverage
    PERM = [(0, 1, 2, 3, 4, 5, 6, 7), (1, 0, 3, 2, 5, 4, 7, 6)]

    in_maps = []
    for c in range(8):
        b, cls = c >> 1, c & 1
        pos = POS[cls]
        perm = PERM[cls]
        # chunk-contiguous: [128, NC, ND*512]
        xt3_np = np.stack(
            [xt3_b[b][:, :, pc * 512:(pc + 1) * 512].reshape(128, ND * 512)
             for pc in perm], axis=1)
        qx_np = np.stack(
            [xt3_b[b][:, :, pos[w] * QW:(pos[w] + 1) * QW].reshape(128, ND * 512)
             for w in (1, 2, 3)], axis=1)
        # validity of the maybe-OOB groups: for slot s the group OOB_JG[s]
        # is valid iff its original chunk index <= pos[s]
        misc_np = np.zeros((128, 16), np.float32)
        for s_ in range(NQ):
            jg = OOB_JG[s_]
            lc = OOB_LC[jg]
            valid = 1.0 if perm[jg] <= pos[s_] else 0.0
            misc_np[:, 4 * lc:4 * lc + 4] = valid
        in_maps.append({
            "xt3": np.ascontiguousarray(xt3_np),
            "qx": np.ascontiguousarray(qx_np),
            "wall": wall_np,
            "misc": misc_np,
        })

    trace = bool(int(os.environ.get("KERNEL_TRACE", "0")))
    res = run_bass_kernel_spmd(nc, in_maps, core_ids=list(range(8)), trace=trace)
    _cache["last_result"] = res

    out = np.zeros((B, S, H), np.float32)
    for c in range(8):
        b, cls = c >> 1, c & 1
        oc = res.results[c]["o"].astype(np.float32)   # [NQ, 65, 512] bf16
        for s_, p in enumerate(POS[cls]):
            num = oc[s_, 0:H, :]          # [64, 512]
            den = oc[s_, H, :]            # [512]
            out[b, p * QW:(p + 1) * QW, :] = (num / den[None, :]).T
    return out


# revision 25
# speedup vs baseline: 1.1400x; 1.0673x over previous
import sys

sys.path.insert(0, "/opt/trn_rl_repo")

import os
import numpy as np
import ml_dtypes

import concourse.bass as bass
import concourse.mybir as mybir
import concourse.tile as tile
from concourse import bacc
from concourse.bass_utils import run_bass_kernel_spmd
from concourse.masks import make_identity

B, S, D, H = 4, 4096, 1024, 64
QW = 512                      # q-chunk width
NQ = 4                        # q-chunk slots per core
POS = [(0, 3, 4, 7), (1, 2, 5, 6)]   # q-chunk positions per core class
T = (8, 16, 24, 32)           # k-tiles (128 wide) per slot in the uniform graph
NKT = S // 128                # 32 k tiles
NC = 8                        # permuted 512-chunks
ND = D // 128                 # 8 d-tiles
DIAG_JG = (0, 3, 4, 7)        # diagonal 512-group per slot (class-independent)
OOB_JG = (1, 2, 5, 6)         # maybe-out-of-bounds group per slot
OOB_LC = {1: 0, 2: 1, 5: 2, 6: 3}   # chunk -> local index in vws2

BF = mybir.dt.bfloat16
F32 = mybir.dt.float32

_cache = {}


def _build():
    nc = bacc.Bacc("TRN2", target_bir_lowering=False, debug=False, num_devices=8)

    # host pre-swizzled inputs: chunk-contiguous so each chunk DMA is 128
    # descriptors of 8KB (descriptor GEN on the sync sequencer paces the
    # input stream, not queue bandwidth)
    xt3 = nc.dram_tensor("xt3", [128, NC, ND * 512], BF, kind="ExternalInput").ap()
    # slot-1/2/3 q-chunk blocks, loaded early so no qproj waits on the
    # main chunk stream
    qx = nc.dram_tensor("qx", [128, 3, ND * 512], BF, kind="ExternalInput").ap()
    wall = nc.dram_tensor("wall", [128, ND * 384], BF, kind="ExternalInput").ap()
    # per-tile validity scalars for the maybe-OOB groups (chunks 1,2,5,6)
    misc = nc.dram_tensor("misc", [128, 16], F32, kind="ExternalInput").ap()
    o = nc.dram_tensor("o", [NQ, H + 1, QW], BF, kind="ExternalOutput").ap()

    with tile.TileContext(nc) as tc:
        with (
            tc.tile_pool(name="persist", bufs=1) as pp,
            tc.tile_pool(name="xin", bufs=1) as xp,
            tc.tile_pool(name="estage", bufs=6) as ep,
            tc.tile_pool(name="vstage", bufs=3) as vsp,
            tc.tile_pool(name="ostage", bufs=2) as osp,
            tc.tile_pool(name="zpsum", bufs=2, space="PSUM") as zp,
            tc.tile_pool(name="opsum", bufs=1, space="PSUM") as op_,
            tc.tile_pool(name="projpsum", bufs=2, space="PSUM") as prp,
            tc.tile_pool(name="vtpsum", bufs=1, space="PSUM") as vtp,
        ):
            # ---- persistent tiles ----
            wall_sb = pp.tile([128, ND * 384], BF, tag="wall")
            misc_sb = pp.tile([128, 16], F32, tag="misc")
            ident = pp.tile([64, 64], BF, tag="ident")
            qT2 = pp.tile([128, NQ * QW], BF, tag="qT2")
            kT2 = pp.tile([128, S], BF, tag="kT2")
            vws = pp.tile([128, NKT * (H + 1)], BF, tag="vws")
            # zero-or-copy of v for the maybe-OOB groups (chunks 1,2,5,6)
            vws2 = pp.tile([128, 16 * (H + 1)], BF, tag="vws2")
            # e-queues: later slots' first pairs exp'd early inside the
            # previous slot's holes; slot-3 pairs 0-7 during attn2
            eq1 = pp.tile([128, 2 * 1024], BF, tag="eq1")
            eq2 = pp.tile([128, 2 * 1024], BF, tag="eq2")
            eq3 = pp.tile([128, 8 * 1024], BF, tag="eq3")
            xtall = xp.tile([128, NC, ND, 512], BF, tag="xtall")
            qxall = xp.tile([128, 3, ND, 512], BF, tag="qxall")

            # ---- input DMAs: one trigger per logical block, consumption order
            # (wall and chunk 0 split by d-halves so the first kv/qproj
            # matmuls start as early as possible)
            nc.sync.dma_start(misc_sb[:], misc[:])
            nc.sync.dma_start(wall_sb[:, 0:4 * 384], wall[:, 0:4 * 384])
            x3v = xtall[:].rearrange("p c d w -> p c (d w)")
            nc.sync.dma_start(x3v[:, 0:1, 0:2048], xt3[:, 0:1, 0:2048])
            nc.sync.dma_start(wall_sb[:, 4 * 384:], wall[:, 4 * 384:])
            nc.sync.dma_start(x3v[:, 0:1, 2048:4096], xt3[:, 0:1, 2048:4096])
            qxv = qxall[:].rearrange("p c d w -> p c (d w)")
            nc.sync.dma_start(qxv[:, 0:1], qx[:, 0:1])
            nc.sync.dma_start(x3v[:, 1:2], xt3[:, 1:2])
            nc.sync.dma_start(qxv[:, 1:3], qx[:, 1:3])
            for c in range(2, NC):
                nc.sync.dma_start(x3v[:, c:c + 1], xt3[:, c:c + 1])

            make_identity(nc, ident[:])
            # hold the PE busy (HAM warm) on the identity tile (available
            # immediately, no DMA dependency) while inputs stream in
            wtile = vtp.tile([128, 64], F32, tag="vt", name="warmps")
            for i in range(90):
                nc.tensor.matmul(wtile[0:64, :], ident[:], ident[:],
                                 start=True, stop=True)
            # big memsets on DVE so the gpsimd queue stays clear for the
            # identity build (gpsimd would delay the PE warmup otherwise)
            nc.vector.memset(vws[:], 1.0)
            nc.vector.memset(vws2[:], 0.0)
            # vws2 ones-row = per-tile validity (0/1 from host)
            v2ones = vws2[:].rearrange("p (t h) -> p t h", h=H + 1)[:, :, H:H + 1]
            nc.vector.tensor_copy(v2ones, misc_sb[:].rearrange(
                "p (t u) -> p t u", u=1))
            # warm the ACT exp table early
            warm = ep.tile([128, 1], BF, tag="warm")
            nc.scalar.activation(warm[:], misc_sb[:, 0:1],
                                 mybir.ActivationFunctionType.Exp)

            _qps = {}

            def qproj_a(w):
                ps = prp.tile([128, 512], F32, tag="proj", name=f"qps{w}")
                _qps[w] = ps
                for d in range(4):
                    if w >= 1:
                        rhs = qxall[:, w - 1, d, :]
                    else:
                        rhs = xtall[:, 0, d, :]
                    nc.tensor.matmul(ps[:], wall_sb[:, d * 384:d * 384 + 128],
                                     rhs, start=(d == 0), stop=False)

            def qproj_b(w, eng=None):
                ps = _qps.pop(w)
                for d in range(4, ND):
                    if w >= 1:
                        rhs = qxall[:, w - 1, d, :]
                    else:
                        rhs = xtall[:, 0, d, :]
                    nc.tensor.matmul(ps[:], wall_sb[:, d * 384:d * 384 + 128],
                                     rhs, start=False, stop=(d == ND - 1))
                # rows 0-63 and 64-127 both hold q^T (duplicated weights)
                if eng == "scalar":
                    nc.scalar.copy(qT2[:, w * QW:(w + 1) * QW], ps[:])
                else:
                    nc.vector.tensor_copy(qT2[:, w * QW:(w + 1) * QW], ps[:])

            def qproj(w):
                qproj_a(w)
                qproj_b(w)

            _kvps = {}
            _kvvst = {}

            def _xev(sc, d, half):
                # even (half=0) or odd (half=1) 128-tiles of chunk sc, d-tile d
                return xtall[:, sc, d, :].rearrange(
                    "p (a b w) -> p a b w", a=2, b=2)[:, :, half, :]

            def kv_mms(sc, half, d0, d1):
                ps = _kvps[sc]
                w0 = 128 if half == 0 else 256
                c0_, c1_ = (0, 256) if half == 0 else (256, 512)
                for d in range(d0, d1):
                    nc.tensor.matmul(ps[:, c0_:c1_],
                                     wall_sb[:, d * 384 + w0:d * 384 + w0 + 128],
                                     _xev(sc, d, half),
                                     start=(d == 0), stop=(d == ND - 1))

            def kv_1(sc):
                # A-half: [wk|wv] weights on the EVEN k-tiles of the chunk
                prp_t = prp.tile([128, 512], F32, tag="proj", name=f"kvps{sc}")
                _kvps[sc] = prp_t
                kv_mms(sc, 0, 0, ND)

            def _cp(eng, dst, src):
                if eng == "scalar":
                    nc.scalar.copy(dst, src)
                else:
                    nc.vector.tensor_copy(dst, src)

            def kv_copyA(sc, eng=None):
                # A-half copies: k of even tiles -> kT2 rows 0:64
                ps = _kvps[sc]
                kdst = kT2[0:64, sc * 512:(sc + 1) * 512].rearrange(
                    "p (a b w) -> p a b w", a=2, b=2)
                _cp(eng, kdst[:, :, 0, :],
                    ps[0:64, 0:256].rearrange("p (t w) -> p t w", w=128))
                vstage = vsp.tile([64, 512], BF, tag="vstage", name=f"vst{sc}")
                _kvvst[sc] = vstage
                vdst = vstage[:].rearrange("p (a b w) -> p a b w", a=2, b=2)
                _cp(eng, vdst[:, :, 0, :],
                    ps[64:128, 0:256].rearrange("p (t w) -> p t w", w=128))

            def kv_2(sc, eng=None):
                # B-half: [wv|wk] weights on the ODD k-tiles -> k lands in
                # partitions 64:128 directly (no kdup matmul needed)
                kv_mms(sc, 1, 0, ND)
                kv_copyA(sc, eng)

            def kv_3(sc, eng=None):
                # B-half copies: k of odd tiles -> kT2 rows 64:128
                ps = _kvps.pop(sc)
                kdst = kT2[64:128, sc * 512:(sc + 1) * 512].rearrange(
                    "p (a b w) -> p a b w", a=2, b=2)
                _cp(eng, kdst[:, :, 1, :],
                    ps[64:128, 256:512].rearrange("p (t w) -> p t w", w=128))
                vstage = _kvvst[sc]
                vdst = vstage[:].rearrange("p (a b w) -> p a b w", a=2, b=2)
                _cp(eng, vdst[:, :, 1, :],
                    ps[0:64, 256:512].rearrange("p (t w) -> p t w", w=128))

            def kv_4(sc):
                vstage = _kvvst.pop(sc)
                # all 4 transposes into one PSUM tile (68-col stride keeps the
                # matmul writes 8B-aligned), then ONE strided copy into vws
                vt4 = vtp.tile([128, 4 * 68], BF, tag="vt", name=f"vt4_{sc}")
                for t in range(4):
                    nc.tensor.transpose(vt4[:, t * 68:t * 68 + 64],
                                        vstage[:, t * 128:(t + 1) * 128],
                                        ident[:])
                kt0 = 4 * sc
                dst = vws[:, kt0 * (H + 1):(kt0 + 4) * (H + 1)]
                vt4v = vt4[:].rearrange("p (t h) -> p t h", h=68)[:, :, 0:H]
                nc.vector.tensor_copy(
                    dst.rearrange("p (t h) -> p t h", h=H + 1)[:, :, 0:H], vt4v)
                if sc in OOB_LC:
                    # scaled copy into vws2 (valid -> v, invalid -> 0)
                    lc = OOB_LC[sc]
                    d2 = vws2[:, lc * 4 * (H + 1):(lc + 1) * 4 * (H + 1)]
                    nc.vector.tensor_scalar(
                        d2.rearrange("p (t h) -> p t h", h=H + 1)[:, :, 0:H],
                        vt4v, misc_sb[:, 4 * lc:4 * lc + 1], None,
                        mybir.AluOpType.mult)

            def kv_chunk(sc):
                kv_1(sc)
                kv_2(sc)
                kv_3(sc)
                kv_4(sc)

            def z_exp(s_, p, e_ap, name):
                """z matmul pair + exp (+ causal mask) for slot s_, pair p."""
                j0, j1 = 2 * p, 2 * p + 1
                z = zp.tile([128, 1024], F32, tag="z", name=f"z{name}")
                # two K=64 matmuls in different PE row groups
                nc.tensor.matmul(z[:, 0:512],
                                 kT2[0:64, j0 * 128:(j0 + 1) * 128],
                                 qT2[0:64, s_ * QW:(s_ + 1) * QW],
                                 start=True, stop=True)
                nc.tensor.matmul(z[:, 512:1024],
                                 kT2[64:128, j1 * 128:(j1 + 1) * 128],
                                 qT2[64:128, s_ * QW:(s_ + 1) * QW],
                                 start=True, stop=True)
                nc.scalar.activation(e_ap, z[:],
                                     mybir.ActivationFunctionType.Exp,
                                     scale=0.125)
                if p // 2 == DIAG_JG[s_]:
                    # causal mask for the diagonal 512-block: keep where
                    # q >= p + 128t (chunk-local), else 0
                    nc.gpsimd.affine_select(
                        out=e_ap, in_=e_ap,
                        compare_op=mybir.AluOpType.is_ge,
                        fill=0.0, base=-256 * (p % 2),
                        channel_multiplier=-1,
                        pattern=[[-128, 2], [1, 512]])

            def pre1(p):
                z_exp(1, p, eq1[:, p * 1024:(p + 1) * 1024], f"pre1_{p}")

            def pre2(p):
                z_exp(2, p, eq2[:, p * 1024:(p + 1) * 1024], f"pre2_{p}")

            def pre3(p):
                z_exp(3, p, eq3[:, p * 1024:(p + 1) * 1024], f"pre3_{p}")

            def attn_slot(s_, inject=(), order=None, npre=0, eq=None,
                          pre_inject=(), osb_engine=None):
                ts_ = T[s_]
                np_ = ts_ // 2   # tile pairs
                inj = dict(inject)
                oob_jg = OOB_JG[s_]
                for th in pre_inject:
                    th()
                ops = op_.tile([H + 1, 512], F32, tag="oacc", name=f"oacc{s_}")
                if order is None:
                    order = list(range(np_))
                _es = {}

                def emit_z(p):
                    if p < npre:
                        _es[p] = eq[:, p * 1024:(p + 1) * 1024]
                    else:
                        e = ep.tile([128, 1024], BF, tag="e", name=f"e{s_}_{p}")
                        _es[p] = e[:]
                        z_exp(s_, p, e[:], f"{s_}_{p}")

                # software pipeline: keep 2 z/exp stages in flight ahead of PV
                emit_z(order[0])
                if np_ > 1:
                    emit_z(order[1])
                for i, p in enumerate(order):
                    for th in inj.pop(i, ()):
                        th()
                    if i + 2 < np_:
                        emit_z(order[i + 2])
                    e_ap = _es.pop(p)
                    j0, j1 = 2 * p, 2 * p + 1
                    jg = p // 2
                    if jg == oob_jg:
                        t0 = 4 * OOB_LC[oob_jg] + 2 * (p - 2 * oob_jg)
                        v0 = vws2[:, t0 * (H + 1):(t0 + 1) * (H + 1)]
                        v1 = vws2[:, (t0 + 1) * (H + 1):(t0 + 2) * (H + 1)]
                    else:
                        v0 = vws[:, j0 * (H + 1):(j0 + 1) * (H + 1)]
                        v1 = vws[:, j1 * (H + 1):(j1 + 1) * (H + 1)]
                    nc.tensor.matmul(ops[:], v0, e_ap[:, 0:512],
                                     start=(i == 0), stop=False)
                    nc.tensor.matmul(ops[:], v1, e_ap[:, 512:1024],
                                     start=False, stop=(i == np_ - 1))
                osb = osp.tile([H + 1, 512], BF, tag="osb", name=f"osb{s_}")
                if osb_engine == "scalar":
                    nc.scalar.copy(osb[:], ops[:])
                else:
                    nc.vector.tensor_copy(osb[:], ops[:])
                nc.sync.dma_start(o[s_], osb[:])

            # wave 0 — chunk-0 ramp: consume the first d-half (first DMA)
            # across both kv halves and qproj before the second half lands.
            # A and B accumulation groups are interleaved, so they must live
            # in DIFFERENT PSUM banks (start_tensor_calc clears has_written
            # at bank granularity): A in prp, B in vtp.
            psA0 = prp.tile([128, 512], F32, tag="proj", name="kvpsA0")
            psB0 = vtp.tile([128, 512], F32, tag="vt", name="kvpsB0")

            def _ramp_mms(ps, half, d0, d1):
                w0 = 128 if half == 0 else 256
                for d in range(d0, d1):
                    nc.tensor.matmul(ps[:, 0:256],
                                     wall_sb[:, d * 384 + w0:d * 384 + w0 + 128],
                                     _xev(0, d, half),
                                     start=(d == 0), stop=(d == ND - 1))

            _ramp_mms(psA0, 0, 0, 4)
            _ramp_mms(psB0, 1, 0, 4)
            qproj_a(0)
            _ramp_mms(psA0, 0, 4, ND)
            _ramp_mms(psB0, 1, 4, ND)
            qproj_b(0, eng="scalar")
            # chunk-0 copies (A data in psA0[:, 0:256], B in psB0[:, 0:256]);
            # ACT is idle this early, so run them there to unclog the DVE
            kd0 = kT2[0:64, 0:512].rearrange("p (a b w) -> p a b w", a=2, b=2)
            nc.scalar.copy(
                kd0[:, :, 0, :],
                psA0[0:64, 0:256].rearrange("p (t w) -> p t w", w=128))
            vst0 = vsp.tile([64, 512], BF, tag="vstage", name="vst0")
            _kvvst[0] = vst0
            vd0 = vst0[:].rearrange("p (a b w) -> p a b w", a=2, b=2)
            nc.scalar.copy(
                vd0[:, :, 0, :],
                psA0[64:128, 0:256].rearrange("p (t w) -> p t w", w=128))
            kd0b = kT2[64:128, 0:512].rearrange("p (a b w) -> p a b w", a=2, b=2)
            nc.scalar.copy(
                kd0b[:, :, 1, :],
                psB0[64:128, 0:256].rearrange("p (t w) -> p t w", w=128))
            nc.scalar.copy(
                vd0[:, :, 1, :],
                psB0[0:64, 0:256].rearrange("p (t w) -> p t w", w=128))
            kv_4(0)
            # chunk-0 pairs of attn0 start immediately; every later
            # kv/qproj stage is injected so no chain exceeds the
            # buffered-exp coverage
            attn_slot(0, inject={
                0: (lambda: kv_1(1), lambda: kv_2(1, eng="scalar"),
                    lambda: kv_3(1, eng="scalar")),
                1: (lambda: kv_4(1), lambda: qproj_a(1)),
                2: (lambda: qproj_b(1),),
                3: (lambda: pre1(0), lambda: pre1(1)),
            })
            attn_slot(1, npre=2, eq=eq1, inject={
                0: (lambda: kv_1(2),),
                1: (lambda: kv_2(2),),
                2: (lambda: kv_3(2), lambda: kv_1(3)),
                3: (lambda: kv_4(2), lambda: kv_2(3)),
                4: (lambda: kv_3(3), lambda: qproj_a(2)),
                5: (lambda: kv_4(3), lambda: qproj_b(2)),
                6: (lambda: pre2(0),),
                7: (lambda: pre2(1),),
            })
            attn_slot(2, npre=2, eq=eq2, inject={
                0: (lambda: qproj_a(3),),
                1: (lambda: qproj_b(3),),
                2: (lambda: kv_1(4), lambda: pre3(0)),
                3: (lambda: kv_2(4), lambda: pre3(1)),
                4: (lambda: kv_3(4), lambda: pre3(2)),
                5: (lambda: kv_4(4), lambda: pre3(3)),
                6: (lambda: kv_1(5), lambda: pre3(4)),
                7: (lambda: kv_2(5), lambda: pre3(5)),
                8: (lambda: kv_3(5), lambda: pre3(6)),
                9: (lambda: kv_4(5), lambda: pre3(7)),
            })
            # slot 3: live pairs (8-15) interleaved with PV-only pre-exp'd
            # pairs (0-7); diag pairs 14/15 moved off the tail
            attn_slot(3, npre=8, eq=eq3, osb_engine="scalar",
                      order=[8, 0, 9, 1, 10, 2, 11, 3,
                             14, 4, 15, 5, 12, 6, 13, 7],
                      inject={
                0: (lambda: kv_1(6),),
                1: (lambda: kv_2(6),),
                2: (lambda: kv_3(6),),
                3: (lambda: kv_4(6),),
                4: (lambda: kv_1(7),),
                5: (lambda: kv_2(7),),
                6: (lambda: kv_3(7),),
                7: (lambda: kv_4(7),),
            })

    nc.compile()
    return nc


def _get_nc():
    if "nc" not in _cache:
        _cache["nc"] = _build()
    return _cache["nc"]


def kernel(x, Wk, Wq, Wv):
    x = np.asarray(x, dtype=np.float32)
    Wk = np.asarray(Wk, dtype=np.float32)
    Wq = np.asarray(Wq, dtype=np.float32)
    Wv = np.asarray(Wv, dtype=np.float32)

    nc = _get_nc()

    # wall[p, d*384 + 0:128]   = Wq[d*128+p, :] | Wq  (duplicated)
    # wall[p, d*384 + 128:256] = Wk | Wv   (A-half, even k-tiles)
    # wall[p, d*384 + 256:384] = Wv | Wk   (B-half, odd k-tiles)
    wqq = np.concatenate([Wq, Wq], axis=1)        # [1024, 128]
    wkv = np.concatenate([Wk, Wv], axis=1)        # [1024, 128]
    wvk = np.concatenate([Wv, Wk], axis=1)        # [1024, 128]
    wboth = np.concatenate([wqq, wkv, wvk], axis=1)   # [1024, 384]
    wall_np = np.ascontiguousarray(
        wboth.reshape(ND, 128, 384).transpose(1, 0, 2).reshape(128, ND * 384)
    ).astype(ml_dtypes.bfloat16)

    xt_b = [np.ascontiguousarray(x[b].T).astype(ml_dtypes.bfloat16)
            for b in range(B)]
    # [D, S] -> [128, ND, S] partition-major swizzle
    xt3_b = [np.ascontiguousarray(xb.reshape(ND, 128, S).transpose(1, 0, 2))
             for xb in xt_b]

    # per-class k-chunk permutation: places each class's q-chunks at the
    # uniform positions (0,3,4,7) while preserving causal prefix coverage
    PERM = [(0, 1, 2, 3, 4, 5, 6, 7), (1, 0, 3, 2, 5, 4, 7, 6)]

    in_maps = []
    for c in range(8):
        b, cls = c >> 1, c & 1
        pos = POS[cls]
        perm = PERM[cls]
        # chunk-contiguous: [128, NC, ND*512]
        xt3_np = np.stack(
            [xt3_b[b][:, :, pc * 512:(pc + 1) * 512].reshape(128, ND * 512)
             for pc in perm], axis=1)
        qx_np = np.stack(
            [xt3_b[b][:, :, pos[w] * QW:(pos[w] + 1) * QW].reshape(128, ND * 512)
             for w in (1, 2, 3)], axis=1)
        # validity of the maybe-OOB groups: for slot s the group OOB_JG[s]
        # is valid iff its original chunk index <= pos[s]
        misc_np = np.zeros((128, 16), np.float32)
        for s_ in range(NQ):
            jg = OOB_JG[s_]
            lc = OOB_LC[jg]
            valid = 1.0 if perm[jg] <= pos[s_] else 0.0
            misc_np[:, 4 * lc:4 * lc + 4] = valid
        in_maps.append({
            "xt3": np.ascontiguousarray(xt3_np),
            "qx": np.ascontiguousarray(qx_np),
            "wall": wall_np,
            "misc": misc_np,
        })

    trace = bool(int(os.environ.get("KERNEL_TRACE", "0")))
    res = run_bass_kernel_spmd(nc, in_maps, core_ids=list(range(8)), trace=trace)
    _cache["last_result"] = res

    out = np.zeros((B, S, H), np.float32)
    for c in range(8):
        b, cls = c >> 1, c & 1
        oc = res.results[c]["o"].astype(np.float32)   # [NQ, 65, 512] bf16
        for s_, p in enumerate(POS[cls]):
            num = oc[s_, 0:H, :]          # [64, 512]
            den = oc[s_, H, :]            # [512]
            out[b, p * QW:(p + 1) * QW, :] = (num / den[None, :]).T
    return out
